# revision 32
# baseline (speedup 1.0000x reference)
"""MoE top-1 routing kernel for 8 TRN2 NeuronCores (expert parallelism).

Self-contained: takes full inputs, shards experts across 8 cores, returns the
full output (host sums the 8 disjoint per-expert partials).

Routing design (v4):
- A tiny AllGather warmup is issued first; it absorbs the CC-channel init and
  the cross-core start barrier while gating runs.
- Gating is token-sharded: each core computes fp32 logits for its own 1024
  tokens (8 accumulating matmuls + PE transposes), then DVE softmax/argmax.
- Each core computes local queue positions for its tokens (one triangular
  matmul + tiny 0/1-matrix matmuls), AllGathers the 8x8 per-(shard, expert)
  counts (256 B) to turn them into global slots, and scatters
  (token_id+1, gate) into a zero-prefilled [E*C, 2] buffer laid out
  partition-major per expert region (8 indirect DMAs).
- One 82 KB AllToAll routes expert region e to core e; the 8 received shard
  contributions are summed on DVE (each slot is written by exactly one shard).
  Empty slots decode to index -1 -> huge sentinel -> dropped by the gather
  bounds check, which also implements capacity dropping exactly like the
  reference.
- FFN in bf16: w2 resident in SBUF, w1 streamed twice over two 640-slot
  halves, fused bias+ReLU on the scalar engine, gate-scaled rows scattered
  into the pre-zeroed output. The FFN runs at the GPIO-throttled tensor
  streaming roofline (~350 us for 2x 10.7 GFLOP of bf16 matmul per core).
"""
import numpy as np
import ml_dtypes
from contextlib import ExitStack

import concourse.bass as bass
import concourse.tile as tile
from concourse import bacc, mybir
from concourse.bass_utils import run_bass_kernel_spmd

dt = mybir.dt

B, S, M, E, DFF = 4, 2048, 1024, 8, 4096
T = B * S
C = int(1.25 * T / E)
P = 128
NT = T // P
MC = M // P
DC = DFF // P
SCN = C // P
HALF = C // 2
TSH = T // E
LT = TSH // P
LE = LT * E
BIG = 1.0e9
EC = E * C

_CACHE = {}


def _build_nc(stage=5):
    nc = bacc.Bacc("TRN2", target_bir_lowering=False, debug=False)

    xTs = nc.dram_tensor("xTs", [M, TSH], dt.float32, kind="ExternalInput")
    xb = nc.dram_tensor("xb", [T, M], dt.bfloat16, kind="ExternalInput")
    wg = nc.dram_tensor("wg", [M, E], dt.float32, kind="ExternalInput")
    w1p = nc.dram_tensor("w1p", [DC, P, MC, P], dt.bfloat16, kind="ExternalInput")
    w2p = nc.dram_tensor("w2p", [P, DC, M], dt.bfloat16, kind="ExternalInput")
    b1v = nc.dram_tensor("b1v", [DFF], dt.float32, kind="ExternalInput")
    b2b = nc.dram_tensor("b2b", [P, M], dt.float32, kind="ExternalInput")
    eiota = nc.dram_tensor("eiota", [P, LT, E], dt.float32, kind="ExternalInput")
    triu = nc.dram_tensor("triu", [P, P], dt.float32, kind="ExternalInput")
    identf = nc.dram_tensor("identf", [P, P], dt.float32, kind="ExternalInput")
    identb = nc.dram_tensor("identb", [P, P], dt.bfloat16, kind="ExternalInput")
    w64d = nc.dram_tensor("w64d", [LE, LE + E], dt.float32, kind="ExternalInput")
    wjd = nc.dram_tensor("wjd", [LE, LE], dt.float32, kind="ExternalInput")
    ecbd = nc.dram_tensor("ecbd", [P, LE], dt.float32, kind="ExternalInput")
    tokp1d = nc.dram_tensor("tokp1d", [P, LT], dt.float32, kind="ExternalInput")
    outd = nc.dram_tensor("out", [T, M], dt.float32, kind="ExternalOutput")

    igd_loc = nc.dram_tensor("igd_loc", [EC, 2], dt.float32)
    igd_rcv = nc.dram_tensor("igd_rcv", [EC, 2], dt.float32)
    cnt_loc = nc.dram_tensor("cnt_loc", [E, 1], dt.float32)
    cnt_all = nc.dram_tensor("cnt_all", [LE, 1], dt.float32, addr_space="Shared")
    wrm_l = nc.dram_tensor("wrm_l", [8, 2], dt.float32)
    wrm_a = nc.dram_tensor("wrm_a", [64, 2], dt.float32, addr_space="Shared")

    with tile.TileContext(nc) as tc, ExitStack() as ctx:
        sb = ctx.enter_context(tc.tile_pool(name="sb", bufs=1))
        sbx = ctx.enter_context(tc.tile_pool(name="sbx", bufs=9))
        sbw1 = ctx.enter_context(tc.tile_pool(name="sbw1", bufs=6))
        sbg = ctx.enter_context(tc.tile_pool(name="sbg", bufs=5))
        sbst = ctx.enter_context(tc.tile_pool(name="sbst", bufs=3))
        sbr = ctx.enter_context(tc.tile_pool(name="sbr", bufs=4))

        wz = sb.tile([8, 2], dt.float32)
        nc.vector.memset(wz[:], 0.0)
        nc.sync.dma_start(wrm_l[:], wz[:])
        nc.gpsimd.collective_compute(
            "AllGather", mybir.AluOpType.bypass,
            ins=[wrm_l[:]], outs=[wrm_a[:]],
            replica_groups=[list(range(E))])

        wgt = sb.tile([P, MC * E], dt.float32)
        nc.sync.dma_start(wgt[:], wg[:].rearrange("(mc p) e -> p mc e", p=P))
        xts = []
        for k in range(MC):
            xt = sbx.tile([P, TSH], dt.float32, tag="xt")
            nc.sync.dma_start(xt[:], xTs[k * P:(k + 1) * P, :])
            xts.append(xt)
        eit = sb.tile([P, LE], dt.float32)
        nc.sync.dma_start(eit[:], eiota[:])
        trit = sb.tile([P, P], dt.float32)
        nc.sync.dma_start(trit[:], triu[:])
        idf = sb.tile([P, P], dt.float32)
        nc.sync.dma_start(idf[:], identf[:])
        w64t = sb.tile([LE, LE + E], dt.float32)
        nc.sync.dma_start(w64t[:], w64d[:])
        wjt = sb.tile([LE, LE], dt.float32)
        nc.sync.dma_start(wjt[:], wjd[:])
        ecbt = sb.tile([P, LE], dt.float32)
        nc.sync.dma_start(ecbt[:], ecbd[:])
        tokp1 = sb.tile([P, LT], dt.float32)
        nc.sync.dma_start(tokp1[:], tokp1d[:])
        idb = sb.tile([P, P], dt.bfloat16)
        nc.sync.dma_start(idb[:], identb[:])
        b1t = sb.tile([P, DC], dt.float32)
        nc.sync.dma_start(b1t[:], b1v[:].rearrange("(d p) -> p d", p=P))
        w2t = sb.tile([P, DC * M], dt.bfloat16)
        if stage >= 5:
            for q in range(4):
                nc.sync.dma_start(
                    w2t[:, q * 8 * M:(q + 1) * 8 * M],
                    w2p[:, q * 8:(q + 1) * 8, :])

        ones1 = sb.tile([1, P], dt.float32)
        nc.gpsimd.memset(ones1[:], 1.0)
        onescol = sb.tile([P, 1], dt.float32)
        nc.gpsimd.memset(onescol[:], 1.0)
        nines = sb.tile([P, LE], dt.float32)
        nc.gpsimd.memset(nines[:], 9.0)
        huget = sb.tile([P, LE], dt.float32)
        nc.gpsimd.memset(huget[:], BIG)
        bigt = sb.tile([P, SCN], dt.float32)
        nc.gpsimd.memset(bigt[:], 1.5e9)
        zpre = sb.tile([P, EC * 2 // P], dt.float32)
        nc.vector.memset(zpre[:], 0.0)
        nc.scalar.dma_start(
            igd_loc[:].rearrange("(p c) two -> p c two", p=P), zpre[:])
        b2t = sb.tile([P, M], dt.float32)
        nc.scalar.dma_start(b2t[:], b2b[:])

        eg_stk = sb.tile([P, LT * 2], dt.float32)
        idx_t = sb.tile([P, SCN], dt.int32)
        gate_f = sb.tile([P, SCN], dt.float32)

        lg_stk = sb.tile([P, LE], dt.float32)
        with tc.tile_pool(name="psg", bufs=4, space="PSUM") as psg:
            lgT = sb.tile([8, TSH], dt.float32)
            for blk in range(TSH // 512):
                pl = psg.tile([8, 512], dt.float32, tag="pl")
                for k in range(MC):
                    nc.tensor.matmul(
                        pl[:], lhsT=wgt[:, k * E:(k + 1) * E],
                        rhs=xts[k][:, blk * 512:(blk + 1) * 512],
                        start=(k == 0), stop=(k == MC - 1))
                nc.vector.tensor_copy(lgT[:, blk * 512:(blk + 1) * 512], pl[:])
            for ti in range(LT):
                pq = psg.tile([P, E], dt.float32, tag="pq")
                nc.tensor.transpose(
                    out=pq[:], in_=lgT[:, ti * P:(ti + 1) * P],
                    identity=idf[:8, :8])
                nc.vector.tensor_copy(lg_stk[:, ti * E:(ti + 1) * E], pq[:])
        lg3 = lg_stk[:].rearrange("p (ti e) -> p ti e", e=E)
        mx_stk = sb.tile([P, LT], dt.float32)
        nc.vector.tensor_reduce(
            out=mx_stk[:], in_=lg3, axis=mybir.AxisListType.X,
            op=mybir.AluOpType.max)
        mxb = mx_stk[:].rearrange("p (ti one) -> p ti one", one=1).to_broadcast([P, LT, E])
        ls = sb.tile([P, LE], dt.float32)
        nc.vector.tensor_tensor(
            out=ls[:].rearrange("p (ti e) -> p ti e", e=E), in0=lg3, in1=mxb,
            op=mybir.AluOpType.subtract)
        ex = sb.tile([P, LE], dt.float32)
        nc.scalar.activation(
            ex[:], ls[:], mybir.ActivationFunctionType.Exp)
        s_stk = sb.tile([P, LT], dt.float32)
        nc.vector.tensor_reduce(
            out=s_stk[:], in_=ex[:].rearrange("p (ti e) -> p ti e", e=E),
            axis=mybir.AxisListType.X, op=mybir.AluOpType.add)
        nc.vector.reciprocal(
            eg_stk[:].rearrange("p (ti two) -> p ti two", two=2)[:, :, 1:2],
            s_stk[:].rearrange("p (ti one) -> p ti one", one=1))
        oh = sb.tile([P, LE], dt.uint8)
        nc.vector.tensor_tensor(
            out=oh[:].rearrange("p (ti e) -> p ti e", e=E), in0=lg3, in1=mxb,
            op=mybir.AluOpType.is_equal)
        msk = sb.tile([P, LE], dt.float32)
        nc.vector.select(msk[:], oh[:], eit[:], nines[:])
        nc.vector.tensor_reduce(
            out=eg_stk[:].rearrange("p (ti two) -> p ti two", two=2)[:, :, 0:1],
            in_=msk[:].rearrange("p (ti e) -> p ti e", e=E),
            axis=mybir.AxisListType.X, op=mybir.AluOpType.min)

        eidx_v = eg_stk[:].rearrange("p (ti two) -> p ti two", two=2)[:, :, 0:1]
        gate_v = eg_stk[:].rearrange("p (ti two) -> p ti two", two=2)[:, :, 1:2]
        mine_all = sb.tile([P, LE], dt.float32)
        nc.vector.tensor_tensor(
            out=mine_all[:].rearrange("p (ti e) -> p ti e", e=E),
            in0=eidx_v.to_broadcast([P, LT, E]),
            in1=eit[:].rearrange("p (ti e) -> p ti e", e=E),
            op=mybir.AluOpType.is_equal)

        offsb = sb.tile([1, LE + E], dt.float32)
        cnt_sb = sb.tile([LE, 1], dt.float32)
        off2 = sb.tile([1, LE], dt.float32)
        with tc.tile_pool(name="ppb", bufs=1, space="PSUM") as ppb:
            pts = ppb.tile([LE, 1], dt.float32, tag="pts")
            nc.tensor.matmul(pts[:], lhsT=mine_all[:], rhs=onescol[:],
                             start=True, stop=True)
            tscol = sb.tile([LE, 1], dt.float32)
            nc.vector.tensor_copy(tscol[:], pts[:])
            poffs = ppb.tile([1, LE + E], dt.float32, tag="poffs")
            nc.tensor.matmul(poffs[:], lhsT=tscol[:], rhs=w64t[:],
                             start=True, stop=True)
            nc.vector.tensor_copy(offsb[:], poffs[:])
            nc.sync.dma_start(
                cnt_loc[:].rearrange("a b -> b a"), offsb[:, LE:LE + E])
            nc.gpsimd.collective_compute(
                "AllGather", mybir.AluOpType.bypass,
                ins=[cnt_loc[:]], outs=[cnt_all[:]],
                replica_groups=[list(range(E))])
            nc.sync.dma_start(cnt_sb[:], cnt_all[:])
            pbase = ppb.tile([1, LE], dt.float32, tag="pbase")
            nc.tensor.matmul(pbase[:], lhsT=cnt_sb[:], rhs=wjt[:],
                             start=True, stop=True)
            nc.vector.tensor_tensor(
                out=off2[:], in0=offsb[:, 0:LE], in1=pbase[:],
                op=mybir.AluOpType.add)
            pall = ppb.tile([P, LE], dt.float32, tag="pall")
            nc.tensor.matmul(pall[:], lhsT=trit[:], rhs=mine_all[:],
                             start=True, stop=False)
            nc.tensor.matmul(pall[:], lhsT=ones1[:], rhs=off2[:],
                             start=False, stop=True)
            slotm1 = sb.tile([P, LE], dt.float32)
            nc.vector.tensor_scalar_add(slotm1[:], pall[:], -1.0)
        mu8 = sb.tile([P, LE], dt.uint8)
        nc.vector.tensor_scalar(
            out=mu8[:], in0=mine_all[:], scalar1=0.5, scalar2=None,
            op0=mybir.AluOpType.is_gt)
        s1 = sb.tile([P, LE], dt.float32)
        nc.vector.select(s1[:], mu8[:], slotm1[:], huget[:])
        cu8 = sb.tile([P, LE], dt.uint8)
        nc.vector.tensor_scalar(
            out=cu8[:], in0=s1[:], scalar1=float(C) - 0.5, scalar2=None,
            op0=mybir.AluOpType.is_lt)
        s2 = sb.tile([P, LE], dt.float32)
        nc.vector.select(s2[:], cu8[:], s1[:], huget[:])
        yf = sb.tile([P, LE], dt.float32)
        nc.vector.tensor_scalar(
            out=yf[:], in0=s2[:], scalar1=1.0 / P, scalar2=None,
            op0=mybir.AluOpType.mult)
        qi = sb.tile([P, LE], dt.int32)
        nc.vector.tensor_copy(qi[:], yf[:])
        qf = sb.tile([P, LE], dt.float32)
        nc.vector.tensor_copy(qf[:], qi[:])
        df = sb.tile([P, LE], dt.float32)
        nc.vector.tensor_tensor(
            out=df[:], in0=yf[:], in1=qf[:], op=mybir.AluOpType.subtract)
        adjf = sb.tile([P, LE], dt.float32)
        nc.vector.tensor_scalar(
            out=adjf[:], in0=df[:], scalar1=0.0, scalar2=None,
            op0=mybir.AluOpType.is_lt)
        nc.vector.tensor_tensor(
            out=qf[:], in0=qf[:], in1=adjf[:], op=mybir.AluOpType.subtract)
        rf = sb.tile([P, LE], dt.float32)
        nc.vector.tensor_scalar(
            out=rf[:], in0=qf[:], scalar1=float(-P), scalar2=None,
            op0=mybir.AluOpType.mult)
        nc.vector.tensor_tensor(
            out=rf[:], in0=s2[:], in1=rf[:], op=mybir.AluOpType.add)
        nc.vector.tensor_scalar(
            out=rf[:], in0=rf[:], scalar1=float(SCN), scalar2=None,
            op0=mybir.AluOpType.mult)
        nc.vector.tensor_tensor(
            out=rf[:], in0=rf[:], in1=qf[:], op=mybir.AluOpType.add)
        nc.vector.tensor_tensor(
            out=rf[:], in0=rf[:], in1=ecbt[:], op=mybir.AluOpType.add)
        rowmin = sb.tile([P, LT], dt.float32)
        nc.vector.tensor_reduce(
            out=rowmin[:].rearrange("p (ti one) -> p ti one", one=1),
            in_=rf[:].rearrange("p (ti e) -> p ti e", e=E),
            axis=mybir.AxisListType.X, op=mybir.AluOpType.min)
        sloti = sb.tile([P, LT], dt.int32)
        nc.vector.tensor_copy(sloti[:], rowmin[:])
        pairs = sb.tile([P, LT * 2], dt.float32)
        nc.vector.tensor_copy(
            pairs[:].rearrange("p (t two) -> p t two", two=2)[:, :, 0:1],
            tokp1[:].rearrange("p (t one) -> p t one", one=1))
        nc.vector.tensor_copy(
            pairs[:].rearrange("p (t two) -> p t two", two=2)[:, :, 1:2],
            gate_v)
        for t in range(LT):
            nc.gpsimd.indirect_dma_start(
                out=igd_loc[:], out_offset=bass.IndirectOffsetOnAxis(
                    ap=sloti[:, t:t + 1], axis=0),
                in_=pairs[:, 2 * t:2 * t + 2], in_offset=None,
                bounds_check=EC - 1, oob_is_err=False)
        nc.gpsimd.collective_compute(
            "AllToAll", mybir.AluOpType.bypass,
            ins=[igd_loc[:]], outs=[igd_rcv[:]],
            replica_groups=[list(range(E))])
        lks = []
        for k in range(E):
            lk = sbr.tile([P, SCN * 2], dt.float32, tag=f"lk{k}")
            nc.sync.dma_start(
                lk[:], igd_rcv[k * C:(k + 1) * C, :].rearrange(
                    "(p c) two -> p c two", p=P))
            lks.append(lk)
        lsum = sb.tile([P, SCN * 2], dt.float32)
        nc.vector.tensor_tensor(
            out=lsum[:], in0=lks[0][:], in1=lks[1][:], op=mybir.AluOpType.add)
        for k in range(2, E):
            nc.vector.tensor_tensor(
                out=lsum[:], in0=lsum[:], in1=lks[k][:],
                op=mybir.AluOpType.add)
        iv = lsum[:].rearrange("p (c two) -> p c two", two=2)[:, :, 0:1]
        gv = lsum[:].rearrange("p (c two) -> p c two", two=2)[:, :, 1:2]
        vu8 = sb.tile([P, SCN], dt.uint8)
        nc.vector.tensor_scalar(
            out=vu8[:], in0=iv, scalar1=0.5, scalar2=None,
            op0=mybir.AluOpType.is_gt)
        idxm1 = sb.tile([P, SCN], dt.float32)
        nc.vector.tensor_scalar_add(
            idxm1[:].rearrange("p (c one) -> p c one", one=1), iv, -1.0)
        idxf = sb.tile([P, SCN], dt.float32)
        nc.vector.select(idxf[:], vu8[:], idxm1[:], bigt[:])
        nc.vector.tensor_copy(idx_t[:], idxf[:])
        nc.vector.tensor_copy(
            gate_f[:].rearrange("p (c one) -> p c one", one=1), gv)

        with (
            tc.tile_pool(name="pstr", bufs=2, space="PSUM") as pstr,
            tc.tile_pool(name="ps1", bufs=2, space="PSUM") as ps1,
            tc.tile_pool(name="ps2", bufs=2, space="PSUM") as ps2,
        ):
            for h in range(2):
                dispT = sb.tile([P, MC * HALF], dt.bfloat16, tag="dispT")
                hT = sb.tile([P, DC * HALF], dt.bfloat16, tag="hT")
                for s5 in range(5):
                    sc = h * 5 + s5
                    gx = sbg.tile([P, M], dt.bfloat16, tag="gx")
                    nc.vector.memset(gx[:], 0.0)
                    nc.gpsimd.indirect_dma_start(
                        out=gx[:], out_offset=None, in_=xb[:],
                        in_offset=bass.IndirectOffsetOnAxis(
                            ap=idx_t[:, sc:sc + 1], axis=0),
                        bounds_check=T - 1, oob_is_err=False)
                    for mm in range(MC):
                        ptg = pstr.tile([P, P], dt.float32, tag="ptg")
                        # transpose as a plain matmul (gx chunk stationary,
                        # identity moving): pipelines with surrounding MMs
                        nc.tensor.matmul(
                            ptg[:], lhsT=gx[:, mm * P:(mm + 1) * P],
                            rhs=idb[:], start=True, stop=True)
                        nc.vector.tensor_copy(
                            dispT[:, mm * HALF + s5 * P:
                                  mm * HALF + (s5 + 1) * P],
                            ptg[:])
                if stage >= 4:
                    for d in range(DC):
                        w1t = sbw1.tile([P, M], dt.bfloat16, tag="w1t")
                        nc.sync.dma_start(w1t[:], w1p[d])
                        pA = ps1.tile([P, 512], dt.float32, tag="pA")
                        pB = ps1.tile([P, P], dt.float32, tag="pB")
                        for mc in range(MC):
                            lhs = w1t[:, mc * P:(mc + 1) * P]
                            nc.tensor.matmul(
                                pA[:], lhsT=lhs,
                                rhs=dispT[:, mc * HALF:mc * HALF + 512],
                                start=(mc == 0), stop=(mc == MC - 1))
                            nc.tensor.matmul(
                                pB[:], lhsT=lhs,
                                rhs=dispT[:, mc * HALF + 512:(mc + 1) * HALF],
                                start=(mc == 0), stop=(mc == MC - 1))
                        nc.scalar.activation(
                            hT[:, d * HALF:d * HALF + 512], pA[:],
                            mybir.ActivationFunctionType.Relu,
                            bias=b1t[:, d:d + 1], scale=1.0)
                        nc.scalar.activation(
                            hT[:, d * HALF + 512:(d + 1) * HALF], pB[:],
                            mybir.ActivationFunctionType.Relu,
                            bias=b1t[:, d:d + 1], scale=1.0)
                if stage >= 5:
                    for s5 in range(5):
                        sc = h * 5 + s5
                        st = sbst.tile([P, M], dt.float32, tag="st")
                        po0 = ps2.tile([P, 512], dt.float32, tag="po")
                        po1 = ps2.tile([P, 512], dt.float32, tag="po")
                        for d in range(DC):
                            lhs = hT[:, d * HALF + s5 * P:d * HALF + (s5 + 1) * P]
                            nc.tensor.matmul(
                                po0[:], lhsT=lhs,
                                rhs=w2t[:, d * M:d * M + 512],
                                start=(d == 0), stop=(d == DC - 1))
                            nc.tensor.matmul(
                                po1[:], lhsT=lhs,
                                rhs=w2t[:, d * M + 512:d * M + 1024],
                                start=(d == 0), stop=(d == DC - 1))
                        for mm, po in ((0, po0), (1, po1)):
                            nc.vector.tensor_tensor(
                                out=st[:, mm * 512:(mm + 1) * 512], in0=po[:],
                                in1=b2t[:, mm * 512:(mm + 1) * 512],
                                op=mybir.AluOpType.add)
                        nc.vector.tensor_scalar_mul(
                            st[:], st[:], gate_f[:, sc:sc + 1])
                        nc.gpsimd.indirect_dma_start(
                            out=outd[:], out_offset=bass.IndirectOffsetOnAxis(
                                ap=idx_t[:, sc:sc + 1], axis=0),
                            in_=st[:], in_offset=None,
                            bounds_check=T - 1, oob_is_err=False)

    nc.compile()
    return nc


def _make_w64():
    w = np.zeros((LE, LE + E), dtype=np.float32)
    for tip in range(LT):
        for ep in range(E):
            r = tip * E + ep
            for ti in range(LT):
                if tip < ti:
                    w[r, ti * E + ep] = 1.0
            w[r, LE + ep] = 1.0
    return w


def _make_wj(j):
    w = np.zeros((LE, LE), dtype=np.float32)
    for jp in range(E):
        for ep in range(E):
            if jp < j:
                r = jp * E + ep
                for ti in range(LT):
                    w[r, ti * E + ep] = 1.0
    return w


def _prep_inputs(x, wg, w1, b1, w2, b2):
    bf16 = ml_dtypes.bfloat16
    tokens = np.ascontiguousarray(x.reshape(T, M)).astype(np.float32)
    xT = np.ascontiguousarray(tokens.T)
    xb = tokens.astype(bf16)
    wgf = np.ascontiguousarray(wg.astype(np.float32))
    eiota = np.broadcast_to(
        np.arange(E, dtype=np.float32), (P, LT, E)).copy()
    triu = np.triu(np.ones((P, P), dtype=np.float32))
    identf = np.eye(P, dtype=np.float32)
    identb = np.eye(P).astype(bf16)
    w64 = _make_w64()
    ecb = np.broadcast_to(
        np.tile(np.arange(E, dtype=np.float32) * C, LT), (P, LE)).copy()
    in_maps = []
    for e in range(E):
        w1e = np.ascontiguousarray(w1[e]).astype(bf16)
        w1pk = np.ascontiguousarray(
            w1e.reshape(MC, P, DC, P).transpose(2, 1, 0, 3))
        w2e = np.ascontiguousarray(w2[e]).astype(bf16)
        w2pk = np.ascontiguousarray(
            w2e.reshape(DC, P, M).transpose(1, 0, 2))
        tokp1 = (e * TSH + np.arange(TSH, dtype=np.float32)
                 .reshape(LT, P).T + 1.0).copy()
        in_maps.append({
            "xTs": np.ascontiguousarray(xT[:, e * TSH:(e + 1) * TSH]),
            "xb": xb, "wg": wgf,
            "w1p": w1pk, "w2p": w2pk,
            "b1v": np.ascontiguousarray(b1[e]).astype(np.float32),
            "b2b": np.tile(np.asarray(b2[e], dtype=np.float32), (P, 1)),
            "eiota": eiota, "triu": triu,
            "identf": identf, "identb": identb,
            "w64d": w64, "wjd": _make_wj(e), "ecbd": ecb,
            "tokp1d": tokp1,
        })
    return in_maps


def kernel(x, wg, w1, b1, w2, b2, _trace=False):
    if "nc" not in _CACHE:
        _CACHE["nc"] = _build_nc()
    nc = _CACHE["nc"]
    in_maps = _prep_inputs(
        np.asarray(x), np.asarray(wg), np.asarray(w1),
        np.asarray(b1), np.asarray(w2), np.asarray(b2))
    res = run_bass_kernel_spmd(nc, in_maps, list(range(E)), trace=_trace)
    _CACHE["last_results"] = res
    full = np.zeros((T, M), dtype=np.float32)
    for e in range(E):
        full += res.results[e]["out"]
    return full.reshape(B, S, M)
